# revision 1
# baseline (speedup 1.0000x reference)
"""CMHSA (1x1-conv multi-head self-attention with a head-mixing 1x1 conv and
instance-norm on the attention maps) as a Trainium2 Bass kernel on 8
NeuronCores.

Reference math (B=4, C=512, T=1024, HEADS=8, hd=64):
  xf = x[b] as [C, T];  q/k/v = W @ xf;  per head h: S_h = q_h^T k_h * hd^-.5
  S'_g = sum_h w_head[g,h] S_h            (head-mixing 1x1 conv)
  A = softmax(S'_g, axis=-1)
  A = instnorm(A) * gamma_g + beta_g      (stats over the whole [T,T] map)
  out = (A @ v_g^T).view(b, t, c) @ w_proj.T + b_proj -> [B, C, H, W]

Transformations:
  * Head-mixing folds into Q: S'_g = (alpha_g . q)^T k with per-channel
    scale alpha_g[o] = w_head[g, o//64].  Each (b, g) map becomes fully
    independent -> 32 maps over 8 cores, 4 maps/core, zero collectives.
  * Logits are ~N(0,1): softmax without max-subtraction is safe.
  * Attention is computed transposed (S^T[T, q]) so the T (softmax) axis is
    the PE contraction axis; softmax row-sums come out of the AV matmul by
    appending one ones-column to the stationary [v_g | 1]: PSUM rows 0-63 =
    v @ E, row 64 = rowsum (pre-broadcast).  A second 1-col ones matmul over
    E^2 gives the per-row sqsum for the variance.
  * gamma/inv_std/beta/b_proj and the constant (beta - a*mu) * sum_T v term
    fold into a host epilogue given per-map sum_q sqsum/rowsum^2, which the
    device emits as a tiny second output.
  * The projection reads the scaled map through a stride-8 access pattern
    that realizes torch's .view(b, t, c) shuffle for free.
"""

import os
import tempfile

import numpy as np

import jax

# Persistent XLA/NEFF compilation cache: without it every launch re-runs the
# client-side BIR->NEFF compile (~0.1s/1000 instructions), which dominates
# repeat-call latency.  Harmless if the dir is not writable.
try:
    _cache_dir = os.environ.get(
        "KERNEL_JAX_CACHE", os.path.join(tempfile.gettempdir(), "jaxcache"))
    os.makedirs(_cache_dir, exist_ok=True)
    jax.config.update("jax_compilation_cache_dir", _cache_dir)
    jax.config.update("jax_persistent_cache_min_compile_time_secs", 0.0)
    jax.config.update("jax_persistent_cache_min_entry_size_bytes", 0)
except Exception:
    pass

import concourse.bass as bass
import concourse.tile as tile
import concourse.mybir as mybir
from concourse import bacc
from concourse.bass_utils import run_bass_kernel_spmd

F32 = mybir.dt.float32
F32R = mybir.dt.float32r

B, C, HH, WW = 4, 512, 32, 32
T = HH * WW          # 1024
HEADS, HD = 8, 64
EPS = 1e-5
SCALE = HD ** -0.5   # 1/8
NCORES = 8
GPC = HEADS // 2     # 4 maps (g values) per core; 2 cores per batch
CC = C // 128        # 4 contraction chunks
TB = T // 128        # 8 T-blocks
MU = 1.0 / T
VG = 65              # per-map stationary width: 64 v-cols + 1 ones-col

_prog_cache = {}


BF16 = mybir.dt.bfloat16


def build_program(reps=1, sq_accum=True, overlap=False, s_reorder=False,
                  s_bf16=False, av_bf16=False, e2_dve=True, depth=2,
                  avb3=False, stage="full"):
    """Build + compile the SPMD Bass program (one NEFF, same for all cores).

    reps>1 repeats the whole compute body (for wall-clock timing via
    wall(reps=R) - wall(reps=1)); input loads run once.
    sq_accum: accumulate E^2 on DVE and contract once per map (vs per-tb
    ones-matmuls on PE).  overlap: let the emission FIFO cross rep
    boundaries.  s_reorder: emit S' matmuls oc-major so back-to-back
    matmuls share a stationary.  s_bf16: k/alpha-q in bf16 for the S'
    matmuls.  av_bf16: vp/e in bf16 for the AV matmuls."""
    key = (reps, sq_accum, overlap, s_reorder, s_bf16, av_bf16, e2_dve,
           depth, avb3, stage)
    if key in _prog_cache:
        return _prog_cache[key]
    lvl = ["qkv", "s", "exp", "e2", "av", "full"].index(stage)

    nc = bacc.Bacc("TRN2", target_bir_lowering=False, debug=False,
                   num_devices=NCORES)

    x_d = nc.dram_tensor("x", [C, T], F32R, kind="ExternalInput")
    wq_d = nc.dram_tensor("wqT", [C, C], F32R, kind="ExternalInput")
    wk_d = nc.dram_tensor("wkT", [C, C], F32R, kind="ExternalInput")
    wv_d = nc.dram_tensor("wvT", [C, GPC * HD], F32R, kind="ExternalInput")
    wp_d = nc.dram_tensor("wpT", [64, 8 * C], F32R, kind="ExternalInput")
    al_d = nc.dram_tensor("alphas", [128, CC * GPC], F32, kind="ExternalInput")
    on_d = nc.dram_tensor("ones", [128, 128], F32R, kind="ExternalInput")
    if av_bf16:
        onb_d = nc.dram_tensor("onesb", [128, 8], BF16, kind="ExternalInput")
    out_d = nc.dram_tensor("out", [GPC * 128, C], F32, kind="ExternalOutput")
    s2_d = nc.dram_tensor("s2", [GPC, 2], F32, kind="ExternalOutput")

    with tile.TileContext(nc) as tc:
        with (
            tc.tile_pool(name="persist", bufs=1) as persist,
            tc.tile_pool(name="qg", bufs=2) as qg_pool,
            tc.tile_pool(name="e", bufs=depth + 1) as e_pool,
            tc.tile_pool(name="e2", bufs=3) as e2_pool,
            tc.tile_pool(name="g", bufs=2) as g_pool,
            tc.tile_pool(name="st", bufs=2) as st_pool,
            tc.tile_pool(name="qkps", bufs=2, space="PSUM") as qk_ps,
            tc.tile_pool(name="avps", bufs=2, space="PSUM") as av_ps,
        ):
            # ---------------- load inputs ----------------
            x_sb = persist.tile([128, CC * T], F32R)   # x[c,t]; chunk cc at cols cc*T
            for cc in range(CC):
                for th in range(2):
                    nc.sync.dma_start(
                        x_sb[:, cc * T + th * 512:cc * T + (th + 1) * 512],
                        x_d[cc * 128:(cc + 1) * 128,
                            th * 512:(th + 1) * 512])
            wq_sb = persist.tile([128, CC * C], F32R)  # w_q.T/8; chunk cc at cols cc*C
            wk_sb = persist.tile([128, CC * C], F32R)
            for w_sb, w_d in ((wq_sb, wq_d), (wk_sb, wk_d)):
                for cc in range(CC):
                    nc.sync.dma_start(w_sb[:, cc * C:(cc + 1) * C],
                                      w_d[cc * 128:(cc + 1) * 128, :])
            wv_sb = persist.tile([128, CC * GPC * HD], F32R)  # this core's v heads
            for cc in range(CC):
                nc.sync.dma_start(
                    wv_sb[:, cc * GPC * HD:(cc + 1) * GPC * HD],
                    wv_d[cc * 128:(cc + 1) * 128, :])
            wp_sb = persist.tile([64, 8 * C], F32R)    # wpT_r[d, jh*512 + c]
            nc.sync.dma_start(wp_sb[:], wp_d[:])
            al_sb = persist.tile([128, CC * GPC], F32)
            nc.sync.dma_start(al_sb[:], al_d[:])
            ones_sb = persist.tile([128, 128], F32R)
            nc.sync.dma_start(ones_sb[:], on_d[:])

            # V' layout: [128, TB * GPC * VG]; group (tb, gi) holds the 64
            # v-columns of map gi for T-block tb plus one ones-column.  The
            # ones-columns are written once, outside the rep loop.
            av_dt = BF16 if av_bf16 else F32R
            vp_sb = persist.tile([128, TB * GPC * VG], av_dt)
            vp_v = vp_sb[:].rearrange("p (t g k) -> p t g k", g=GPC, k=VG)
            for tb in range(TB):
                if av_bf16:
                    nc.sync.dma_start(
                        vp_v[:, tb, :, 64:65],
                        bass.AP(tensor=onb_d, offset=0,
                                ap=[[8, 128], [0, GPC], [1, 1]]))
                else:
                    nc.sync.dma_start(
                        vp_v[:, tb, :, 64:65],
                        bass.AP(tensor=on_d, offset=0,
                                ap=[[128, 128], [0, GPC], [1, 1]]))

            # A single emission FIFO keeps every non-QK chunk of work
            # (AV matmuls, epilogue, projection) trailing ~2 steps behind
            # the QK stream — across map AND rep boundaries — so the PE's
            # static order always has QK matmuls to chew while ACT (exp)
            # and the rowsum-reciprocal chain catch up.
            from collections import deque
            todo = deque()

            def drain(keep):
                while len(todo) > keep:
                    todo.popleft()()

            for _rep in range(reps):
                # ---------------- Q, K = W @ x ----------------
                q_sb = persist.tile([128, CC * T], F32R)   # Q[o,t]; chunk ob at cols ob*T
                k_sb = persist.tile([128, CC * T], BF16 if s_bf16 else F32R)
                for w_sb, dst in ((wq_sb, q_sb), (wk_sb, k_sb)):
                    for ob in range(4):
                        ps = qk_ps.tile([128, 1024], F32, tag="mmps", name="qkv_ps")
                        for th in range(2):
                            for cc in range(CC):
                                nc.tensor.matmul(
                                    ps[:, th * 512:(th + 1) * 512],
                                    (w_sb[:, cc * C + ob * 128:
                                            cc * C + (ob + 1) * 128]),
                                    (x_sb[:, cc * T + th * 512:
                                            cc * T + th * 512 + 512]),
                                    start=(cc == 0), stop=(cc == CC - 1))
                        nc.scalar.copy(dst[:, ob * T:(ob + 1) * T], ps[:])
                        drain(2)

                # ---------------- V' v-columns per T-block ----------------
                for tb in range(TB):
                    ps = qk_ps.tile([128, 1024], F32, tag="mmps", name="vt_ps")
                    for cc in range(CC):
                        nc.tensor.matmul(
                            ps[:, 0:GPC * HD],
                            (x_sb[:, cc * T + tb * 128:cc * T + (tb + 1) * 128]),
                            (wv_sb[:, cc * GPC * HD:(cc + 1) * GPC * HD]),
                            start=(cc == 0), stop=(cc == CC - 1))
                    nc.vector.tensor_copy(
                        vp_v[:, tb, :, 0:64],
                        ps[:, 0:GPC * HD].rearrange("p (g k) -> p g k", k=64))
                    drain(2)

                def emit_qg(gi):
                    qg_sb = qg_pool.tile([128, CC * T],
                                         BF16 if s_bf16 else F32R, tag="qg",
                                         name=f"qg{gi}")
                    for cc in range(CC):
                        nc.vector.tensor_scalar_mul(
                            qg_sb[:, cc * T:(cc + 1) * T],
                            q_sb[:, cc * T:(cc + 1) * T],
                            al_sb[:, cc * GPC + gi:cc * GPC + gi + 1])
                    return qg_sb

                def emit_av(gi, tb, avs, e_t, sqs=None, e2_t=None):
                    for qh in range(2):
                        # rows 0-63: v @ E; row 64: rowsum of E
                        nc.tensor.matmul(
                            avs[qh][0:VG, :],
                            vp_v[:, tb, gi, 0:VG],
                            e_t[:, qh * 512:(qh + 1) * 512],
                            start=(tb == 0), stop=(tb == TB - 1))
                        if sqs is not None:
                            nc.tensor.matmul(
                                sqs[qh][0:1, :],
                                ones_sb[:, 0:1],
                                e2_t[:, qh * 512:(qh + 1) * 512],
                                start=(tb == 0), stop=(tb == TB - 1))

                def emit_sq(e2a):
                    # sqs[qh][0, q] = sum_t E^2: e2a holds the tb-partial
                    # sums, one ones-column contracts the 128 partitions.
                    sqs = tuple(av_ps.tile([128, 512], F32, tag="sq",
                                           name=f"sq{qh}",
                                           bufs=1 if avb3 else 2)
                                for qh in range(2))
                    for qh in range(2):
                        nc.tensor.matmul(
                            sqs[qh][0:1, :],
                            ones_sb[:, 0:1],
                            e2a[:, qh * 512:(qh + 1) * 512],
                            start=True, stop=True)
                    return sqs

                def emit_epilogue(gi, avs, sqs, g_sb):
                    # Copy av/sq PSUM to SBUF immediately (releases the
                    # accumulator banks before the slow reciprocal chain).
                    s2_t = st_pool.tile([128, 2], F32, tag="s2_t",
                                        name="s2_t")
                    avc, sqc = [], []
                    for qh in range(2):
                        a_sb = st_pool.tile([128, 512], F32, tag="avc",
                                            name="a_sb", bufs=3)
                        nc.vector.tensor_copy(a_sb[0:VG, :], avs[qh][0:VG, :])
                        q_sbuf = st_pool.tile([128, 512], F32, tag="sqc",
                                              name="q_sbuf", bufs=3)
                        nc.vector.tensor_copy(q_sbuf[0:1, :],
                                              sqs[qh][0:1, :])
                        avc.append(a_sb)
                        sqc.append(q_sbuf)
                    for qh in range(2):
                        # r = 1/rowsum via exp(-ln .) on ACT (the pinned
                        # table set has both), broadcast to partitions 0..63
                        # with a 1-row fp32 matmul against a 64-col ones
                        # stationary (contraction dim 1 at partition 64).
                        lnr = st_pool.tile([128, 512], F32, tag="lnr",
                                           name="lnr")
                        nc.scalar.activation(lnr[64:65, :], avc[qh][64:65, :],
                                             mybir.ActivationFunctionType.Ln)
                        rrow = st_pool.tile([128, 512], F32, tag="rrow",
                                            name="rrow")
                        nc.scalar.activation(rrow[64:65, :], lnr[64:65, :],
                                             mybir.ActivationFunctionType.Exp,
                                             scale=-1.0)
                        rb_ps = av_ps.tile([128, 512], F32, tag="sq",
                                           name="rb_ps",
                                           bufs=1 if avb3 else 2)
                        nc.tensor.matmul(
                            rb_ps[0:64, :],
                            ones_sb[64:65, 0:64].bitcast(F32),
                            rrow[64:65, :],
                            start=True, stop=True)
                        # G = (v@E) * r
                        nc.vector.tensor_tensor(
                            g_sb[:, qh * 512:(qh + 1) * 512],
                            avc[qh][0:64, :], rb_ps[0:64, :],
                            mybir.AluOpType.mult)
                        # s2[qh] = sum_q sqsum * r^2
                        u = st_pool.tile([128, 512], F32, tag="u", name="u")
                        nc.vector.tensor_tensor(u[0:1, :], sqc[qh][0:1, :],
                                                rb_ps[0:1, :],
                                                mybir.AluOpType.mult)
                        u2 = st_pool.tile([128, 512], F32, tag="u", name="u2")
                        nc.vector.tensor_tensor(u2[0:1, :], u[0:1, :],
                                                rb_ps[0:1, :],
                                                mybir.AluOpType.mult)
                        nc.vector.reduce_sum(s2_t[0:1, qh:qh + 1],
                                             u2[0:1, :],
                                             axis=mybir.AxisListType.X)
                    nc.sync.dma_start(s2_d[gi:gi + 1, :], s2_t[0:1, :])

                def emit_proj(gi, g_sb):
                    # out^T[i,c] = sum_jh sum_d G[d, 8i+jh] * wp[d, jh*512+c]
                    g_v = g_sb[:].rearrange("p (i s) -> p s i", s=8)
                    p_ps = av_ps.tile([128, 512], F32, tag="av", name="p_ps",
                                      bufs=3 if avb3 else 2)
                    for jh in range(8):
                        nc.tensor.matmul(p_ps[:], g_v[:, jh, :],
                                         wp_sb[:, jh * C:(jh + 1) * C],
                                         start=(jh == 0), stop=(jh == 7))
                    stage = st_pool.tile([128, 512], F32, tag="stage",
                                         name="stage", bufs=2)
                    nc.scalar.copy(stage[:], p_ps[:])
                    nc.sync.dma_start(out_d[gi * 128:(gi + 1) * 128, :],
                                      stage[:])

                qg_next = emit_qg(0)
                for gi in range(GPC):
                    qg_sb = qg_next
                    avs = tuple(av_ps.tile([128, 512], F32, tag="av",
                                           name=f"av{qh}",
                                           bufs=3 if avb3 else 2)
                                for qh in range(2))
                    if sq_accum:
                        e2a = e2_pool.tile([128, 1024], F32R, tag="e2a",
                                           name="e2a", bufs=2)
                        sqs = None
                    else:
                        e2a = None
                        sqs = tuple(av_ps.tile([128, 512], F32, tag="sq",
                                               name=f"sq{qh}")
                                    for qh in range(2))

                    for tb in range(TB):
                        if tb == 4 and gi + 1 < GPC:
                            qg_next = emit_qg(gi + 1)
                        if lvl < 1:
                            continue
                        s_ps = qk_ps.tile([128, 1024], F32, tag="mmps",
                                          name="s_ps")
                        if s_reorder:
                            order = [(qh, oc) for oc in range(CC)
                                     for qh in range(2)]
                        else:
                            order = [(qh, oc) for qh in range(2)
                                     for oc in range(CC)]
                        for qh, oc in order:
                            nc.tensor.matmul(
                                s_ps[:, qh * 512:(qh + 1) * 512],
                                k_sb[:, oc * T + tb * 128:
                                     oc * T + (tb + 1) * 128],
                                qg_sb[:, oc * T + qh * 512:
                                      oc * T + qh * 512 + 512],
                                start=(oc == 0), stop=(oc == CC - 1))
                        if lvl < 2:
                            continue
                        e_t = e_pool.tile([128, 1024], av_dt)
                        nc.scalar.activation(e_t[:], s_ps[:],
                                             mybir.ActivationFunctionType.Exp)
                        if lvl < 3:
                            continue
                        if sq_accum and tb == 0:
                            # DVE owns the E^2 running sum; PE only contracts
                            # it once per map in emit_sq.
                            nc.vector.tensor_mul(e2a[:], e_t[:], e_t[:])
                        else:
                            e2_t = e2_pool.tile([128, 1024], F32R)
                            if e2_dve or tb % 2 == 0:
                                nc.vector.tensor_mul(e2_t[:], e_t[:], e_t[:])
                            else:
                                nc.scalar.activation(
                                    e2_t[:], s_ps[:],
                                    mybir.ActivationFunctionType.Exp,
                                    scale=2.0)
                            if sq_accum:
                                nc.vector.tensor_add(e2a[:], e2a[:], e2_t[:])
                        if lvl < 4:
                            continue
                        if sq_accum:
                            todo.append(lambda gi=gi, tb=tb, a=avs, e=e_t:
                                        emit_av(gi, tb, a, e))
                        else:
                            todo.append(lambda gi=gi, tb=tb, a=avs, s=sqs,
                                        e=e_t, e2=e2_t:
                                        emit_av(gi, tb, a, e, s, e2))
                        while len(todo) > depth:
                            todo.popleft()()
                    if lvl < 5:
                        continue
                    g_sb = g_pool.tile([64, 1024], F32R)
                    if sq_accum:
                        todo.append(lambda e2a=e2a, gi=gi, a=avs, g=g_sb:
                                    emit_epilogue(gi, a, emit_sq(e2a), g))
                    else:
                        todo.append(lambda gi=gi, a=avs, s=sqs, g=g_sb:
                                    emit_epilogue(gi, a, s, g))
                    todo.append(lambda gi=gi, g=g_sb: emit_proj(gi, g))
                if not overlap:
                    while todo:
                        todo.popleft()()
            while todo:
                todo.popleft()()

    _pin_act_table(nc)
    nc.compile()
    _prog_cache[key] = nc
    return nc


def _pin_act_table(nc):
    """Make Exp/Ln/Copy resolvable only via natural_log_exp_and_others so the
    act-table-load pass keeps one set resident (no per-map Exp<->Ln table
    thrash).  Instance-level override; set ids keep matching act_info.json."""
    import bass_rust as _bass_rust
    from concourse.hw_specs import get_activation_tables

    keep = "natural_log_exp_and_others"
    af = mybir.ActivationFunctionType
    ours = {af.Exp, af.Ln, af.Copy, af.Identity}

    def patched_pass():
        has_activation = any(
            isinstance(i, mybir.InstActivation)
            for b in nc.main_func.blocks for i in b.instructions)
        if not has_activation:
            return
        tables = get_activation_tables(nc.m.arch)
        if keep in tables and ours <= set(tables[keep]):
            tables = {name: (fns if name == keep else set(fns) - ours)
                      for name, fns in tables.items()}
        _bass_rust.insert_act_table_loads(nc, list(tables.items()))

    nc.insert_act_table_loads = patched_pass


def _host_prep(x, w_q, w_k, w_v, w_head, in_gamma, in_beta, w_proj, b_proj):
    """Build the 8 per-core input maps (all fp32 numpy)."""
    x = np.asarray(x, dtype=np.float32)
    w_q = np.asarray(w_q, dtype=np.float32)
    w_k = np.asarray(w_k, dtype=np.float32)
    w_v = np.asarray(w_v, dtype=np.float32)
    w_head = np.asarray(w_head, dtype=np.float32)

    wqT = np.ascontiguousarray(w_q.T) * np.float32(SCALE)
    wkT = np.ascontiguousarray(w_k.T)
    wpT_r = np.ascontiguousarray(
        np.asarray(w_proj, dtype=np.float32)
        .T.reshape(8, 64, C).transpose(1, 0, 2).reshape(64, 8 * C))

    in_maps = []
    p = np.arange(128)
    for core in range(NCORES):
        b = core // 2
        g0 = (core % 2) * GPC
        xc = np.ascontiguousarray(x[b].reshape(C, T))
        wvT = np.ascontiguousarray(w_v.T[:, g0 * HD:(g0 + GPC) * HD])
        al = np.empty((128, CC * GPC), dtype=np.float32)
        for cc in range(CC):
            for gi in range(GPC):
                al[:, cc * GPC + gi] = w_head[g0 + gi, cc * 2 + p // 64]
        import ml_dtypes
        in_maps.append({
            "x": xc, "wqT": wqT, "wkT": wkT, "wvT": wvT,
            "wpT": wpT_r, "alphas": al,
            "ones": np.ones((128, 128), dtype=np.float32),
            "onesb": np.ones((128, 8), dtype=ml_dtypes.bfloat16),
        })
    return in_maps


def _host_finish(results, x, w_v, w_head, in_gamma, in_beta, w_proj, b_proj):
    in_gamma = np.asarray(in_gamma, dtype=np.float32)
    in_beta = np.asarray(in_beta, dtype=np.float32)
    w_proj = np.asarray(w_proj, dtype=np.float32)
    b_proj = np.asarray(b_proj, dtype=np.float32)
    w_v = np.asarray(w_v, dtype=np.float32)
    x = np.asarray(x, dtype=np.float32)

    # collapsed_wp[d, c] = sum_jh w_proj[c, jh*64+d]
    collapsed_wp = w_proj.reshape(C, 8, 64).sum(axis=1).T   # [64, C]
    out = np.empty((B, C, T), dtype=np.float32)
    for core in range(NCORES):
        b = core // 2
        g0 = (core % 2) * GPC
        dev = results[core]["out"]              # [512 i, 512 c]
        s2 = results[core]["s2"].sum(axis=1)    # [GPC]
        sv = w_v @ x[b].reshape(C, T).sum(axis=1)   # [C]
        for gi in range(GPC):
            g = g0 + gi
            var = s2[gi] / float(T * T) - MU * MU
            a = in_gamma[g] / np.sqrt(var + EPS)
            cs = in_beta[g] - a * MU
            bias2 = collapsed_wp.T @ sv[g * HD:(g + 1) * HD]   # [C]
            blk = dev[gi * 128:(gi + 1) * 128, :]              # [128 i, C]
            full = a * blk + (cs * bias2 + b_proj)[None, :]
            out[b, :, g * 128:(g + 1) * 128] = full.T
    return out.reshape(B, C, HH, WW)


_runner_cache = {}


def _make_runner(nc):
    """One-time jax.jit of the SPMD launch for ``nc`` (the per-call closure
    inside run_bass_kernel_spmd defeats jax's jit cache, costing a re-trace
    plus compile-cache lookup on every call)."""
    if id(nc) in _runner_cache:
        return _runner_cache[id(nc)]

    from concourse import bass2jax as b2j
    import concourse.mybir as _mybir

    b2j.install_neuronx_cc_hook()
    partition_name = (nc.partition_id_tensor.name
                      if nc.partition_id_tensor else None)
    in_names, out_names, out_avals, zero_shapes = [], [], [], []
    for alloc in nc.m.functions[0].allocations:
        if not isinstance(alloc, _mybir.MemoryLocationSet):
            continue
        name = alloc.memorylocations[0].name
        if alloc.kind == "ExternalInput":
            if name != partition_name:
                in_names.append(name)
        elif alloc.kind == "ExternalOutput":
            shape = tuple(alloc.tensor_shape)
            dtype = _mybir.dt.np(alloc.dtype)
            out_names.append(name)
            out_avals.append(jax.core.ShapedArray(shape, dtype))
            zero_shapes.append((shape, dtype))
    n_params = len(in_names)
    all_names = in_names + out_names + (
        [partition_name] if partition_name else [])

    def _body(*args):
        operands = list(args)
        if partition_name is not None:
            operands.append(b2j.partition_id_tensor())
        return tuple(b2j._bass_exec_p.bind(
            *operands, out_avals=tuple(out_avals), in_names=tuple(all_names),
            out_names=tuple(out_names), lowering_input_output_aliases=(),
            sim_require_finite=True, sim_require_nnan=True, nc=nc))

    from jax.experimental.shard_map import shard_map
    from jax.sharding import Mesh, NamedSharding, PartitionSpec
    devices = jax.devices()[:NCORES]
    mesh = Mesh(np.asarray(devices), ("core",))
    nin = n_params + len(out_names)
    sharded = jax.jit(
        shard_map(_body, mesh=mesh, in_specs=(PartitionSpec("core"),) * nin,
                  out_specs=(PartitionSpec("core"),) * len(out_names),
                  check_rep=False),
        keep_unused=True)
    shard = NamedSharding(mesh, PartitionSpec("core"))
    dev_cache = {}

    def run(in_maps):
        # Keep inputs (and the zero output-seed buffers) device-resident
        # across calls with identical host arrays: repeat launches then skip
        # the multi-MB host->device transfer entirely.
        key = id(in_maps)
        if dev_cache.get("key") != key:
            concat_in = [
                np.concatenate([np.asarray(m[name]) for m in in_maps], axis=0)
                for name in in_names]
            dev_cache["in"] = [jax.device_put(a, shard) for a in concat_in]
            dev_cache["zero"] = [
                jax.device_put(np.zeros((NCORES * s[0], *s[1:]), d), shard)
                for s, d in zero_shapes]
            dev_cache["key"] = key
        outs = sharded(*dev_cache["in"], *dev_cache["zero"])
        return [
            {name: np.asarray(outs[i]).reshape(NCORES, *out_avals[i].shape)[c]
             for i, name in enumerate(out_names)}
            for c in range(NCORES)]

    _runner_cache[id(nc)] = run
    return run


def _run(inputs, trace=False, reps=1):
    nc = build_program(reps)
    in_maps = _host_prep(**inputs)
    results = _make_runner(nc)(in_maps)
    out = _host_finish(results, inputs["x"], inputs["w_v"],
                       inputs["w_head"], inputs["in_gamma"],
                       inputs["in_beta"], inputs["w_proj"], inputs["b_proj"])
    return out, results


def kernel(**inputs):
    out, _ = _run(inputs, trace=False)
    return out



# revision 2
# speedup vs baseline: 1.4707x; 1.4707x over previous
"""CMHSA (1x1-conv multi-head self-attention with head-mixing conv and
instance-norm on the attention maps) as a Trainium2 Bass kernel on 8
NeuronCores.  Data-parallel: 2 cores per batch, 4 head-maps per core
(head-mixing folded into per-map channel scales alpha_g), attention
computed transposed so softmax sums fall out of the AV matmul.

Same math/transformations as kernel.py, restructured for engine balance:
  * Device emits RAW per-row rowsum and sqsum (DMA'd straight out of
    PSUM/SBUF); the s2 = sum_q sqsum*r^2 reduction and the variance ->
    (a, cs) fold happen on host.  Kills the u/u2/reduce DVE chain.
  * E^2 squares run on ACT as exp(2s) re-read from the S' PSUM; the
    running-sum accumulation is split into two chains (even tb on DVE,
    odd tb on Pool) contracted with 2 ones-matmuls each per map.
  * PSUM->SBUF copies spread across ACT/DVE so no engine exceeds the PE's
    ~97us of matmul work.
"""

import os
import tempfile

import numpy as np

import jax

try:
    _cache_dir = os.environ.get(
        "KERNEL_JAX_CACHE", os.path.join(tempfile.gettempdir(), "jaxcache"))
    os.makedirs(_cache_dir, exist_ok=True)
    jax.config.update("jax_compilation_cache_dir", _cache_dir)
    jax.config.update("jax_persistent_cache_min_compile_time_secs", 0.0)
    jax.config.update("jax_persistent_cache_min_entry_size_bytes", 0)
except Exception:
    pass

import concourse.bass as bass
import concourse.tile as tile
import concourse.mybir as mybir
from concourse import bacc
from concourse.bass_utils import run_bass_kernel_spmd

F32 = mybir.dt.float32
F32R = mybir.dt.float32r
BF16 = mybir.dt.bfloat16

B, C, HH, WW = 4, 512, 32, 32
T = HH * WW          # 1024
HEADS, HD = 8, 64
EPS = 1e-5
SCALE = HD ** -0.5   # 1/8
NCORES = 8
GPC = HEADS // 2     # 4 maps (g values) per core; 2 cores per batch
CC = C // 128        # 4 contraction chunks
TB = T // 128        # 8 T-blocks
MU = 1.0 / T
VG = 65              # per-map stationary width: 64 v-cols + 1 ones-col

_prog_cache = {}


def build_program(reps=1, overlap=True, e2_sq="dve", e2_acc="dve",
                  depth=3, s_bf16=True, av_bf16=True, rb_bf16=True,
                  proj2=True, wide=False, reorder=False, stage="full"):
    """e2_sq: where E^2 comes from: "act" (exp(2s) on ACT), "dve" (square on
    DVE), "alt" (alternate per tb).  e2_acc: who owns the running sums:
    "split" (even chain DVE, odd chain Pool), "dve", "pool", "pe" (per-tb
    ones-matmul accumulation in PSUM, no chains)."""
    key = (reps, overlap, e2_sq, e2_acc, depth, s_bf16, av_bf16, rb_bf16,
           proj2, wide, reorder, stage)
    if key in _prog_cache:
        return _prog_cache[key]
    lvl = ["qkv", "s", "exp", "e2", "av", "full"].index(stage)

    nc = bacc.Bacc("TRN2", target_bir_lowering=False, debug=False,
                   num_devices=NCORES)

    x_d = nc.dram_tensor("x", [C, T], F32R, kind="ExternalInput")
    wq_d = nc.dram_tensor("wqT", [C, C], F32R, kind="ExternalInput")
    wk_d = nc.dram_tensor("wkT", [C, C], F32R, kind="ExternalInput")
    wv_d = nc.dram_tensor("wvT", [C, GPC * HD], F32R, kind="ExternalInput")
    wp_d = nc.dram_tensor("wpT", [64, 8 * C], F32R, kind="ExternalInput")
    wp2_d = nc.dram_tensor("wpT2", [128, 4 * C], F32R, kind="ExternalInput")
    al_d = nc.dram_tensor("alphas", [128, CC * GPC], F32, kind="ExternalInput")
    on_d = nc.dram_tensor("ones", [128, 128], F32R, kind="ExternalInput")
    onb_d = nc.dram_tensor("onesb", [128, 64], BF16, kind="ExternalInput")
    out_d = nc.dram_tensor("out", [GPC * 128, C], F32, kind="ExternalOutput")
    # stats per map g: row g = [rowsum qh0 | rowsum qh1 | sq qh0 | sq qh1]
    st_d = nc.dram_tensor("stats", [GPC, 2048], F32, kind="ExternalOutput")

    with tile.TileContext(nc) as tc:
        with (
            tc.tile_pool(name="persist", bufs=1) as persist,
            tc.tile_pool(name="qg", bufs=2) as qg_pool,
            tc.tile_pool(name="e", bufs=depth + 1) as e_pool,
            tc.tile_pool(name="e2", bufs=3) as e2_pool,
            tc.tile_pool(name="g", bufs=2) as g_pool,
            tc.tile_pool(name="st", bufs=2) as st_pool,
            tc.tile_pool(name="qkps", bufs=2, space="PSUM") as qk_ps,
            tc.tile_pool(name="avps", bufs=2, space="PSUM") as av_ps,
        ):
            # ---------------- load inputs ----------------
            x_sb = persist.tile([128, CC * T], F32R)
            for cc in range(CC):
                for th in range(2):
                    nc.sync.dma_start(
                        x_sb[:, cc * T + th * 512:cc * T + (th + 1) * 512],
                        x_d[cc * 128:(cc + 1) * 128,
                            th * 512:(th + 1) * 512])
            wq_sb = persist.tile([128, CC * C], F32R)
            wk_sb = persist.tile([128, CC * C], F32R)
            for w_sb, w_d in ((wq_sb, wq_d), (wk_sb, wk_d)):
                for cc in range(CC):
                    nc.sync.dma_start(w_sb[:, cc * C:(cc + 1) * C],
                                      w_d[cc * 128:(cc + 1) * 128, :])
            wv_sb = persist.tile([128, CC * GPC * HD], F32R)
            for cc in range(CC):
                nc.sync.dma_start(
                    wv_sb[:, cc * GPC * HD:(cc + 1) * GPC * HD],
                    wv_d[cc * 128:(cc + 1) * 128, :])
            if proj2:
                wp_sb = persist.tile([128, 4 * C], F32R)
                nc.sync.dma_start(wp_sb[:], wp2_d[:])
            else:
                wp_sb = persist.tile([64, 8 * C], F32R)
                nc.sync.dma_start(wp_sb[:], wp_d[:])
            al_sb = persist.tile([128, CC * GPC], F32)
            nc.sync.dma_start(al_sb[:], al_d[:])
            ones_sb = persist.tile([128, 128], F32R)
            nc.sync.dma_start(ones_sb[:], on_d[:])
            onb_sb = persist.tile([128, 64], BF16)
            nc.sync.dma_start(onb_sb[:], onb_d[:])

            av_dt = BF16 if av_bf16 else F32R
            vp_sb = persist.tile([128, TB * GPC * VG], av_dt)
            vp_v = vp_sb[:].rearrange("p (t g k) -> p t g k", g=GPC, k=VG)
            for tb in range(TB):
                if av_bf16:
                    nc.sync.dma_start(
                        vp_v[:, tb, :, 64:65],
                        bass.AP(tensor=onb_d, offset=0,
                                ap=[[64, 128], [0, GPC], [1, 1]]))
                else:
                    nc.sync.dma_start(
                        vp_v[:, tb, :, 64:65],
                        bass.AP(tensor=on_d, offset=0,
                                ap=[[128, 128], [0, GPC], [1, 1]]))

            from collections import deque
            todo = deque()

            def drain(keep):
                while len(todo) > keep:
                    todo.popleft()()

            for _rep in range(reps):
                # ---------------- Q, K = W @ x ----------------
                q_sb = persist.tile([128, CC * T],
                                    BF16 if s_bf16 else F32R)
                k_sb = persist.tile([128, CC * T], BF16 if s_bf16 else F32R)
                for w_sb, dst in ((wq_sb, q_sb), (wk_sb, k_sb)):
                    for ob in range(4):
                        ps = qk_ps.tile([128, 1024], F32, tag="mmps",
                                        name="qkv_ps")
                        if wide:
                            for cc in range(CC):
                                nc.tensor.matmul(
                                    ps[:, 0:1024],
                                    (w_sb[:, cc * C + ob * 128:
                                            cc * C + (ob + 1) * 128]),
                                    (x_sb[:, cc * T:cc * T + 1024]),
                                    start=(cc == 0), stop=(cc == CC - 1))
                        else:
                            qk_order = ([(th, cc) for cc in range(CC)
                                         for th in range(2)] if reorder else
                                        [(th, cc) for th in range(2)
                                         for cc in range(CC)])
                            for th, cc in qk_order:
                                nc.tensor.matmul(
                                    ps[:, th * 512:(th + 1) * 512],
                                    (w_sb[:, cc * C + ob * 128:
                                            cc * C + (ob + 1) * 128]),
                                    (x_sb[:, cc * T + th * 512:
                                            cc * T + th * 512 + 512]),
                                    start=(cc == 0), stop=(cc == CC - 1))
                        nc.scalar.copy(dst[:, ob * T:(ob + 1) * T], ps[:])
                        drain(2)

                # ---------------- V' v-columns per T-block ----------------
                for tb in range(TB):
                    ps = qk_ps.tile([128, 1024], F32, tag="mmps",
                                    name="vt_ps")
                    for cc in range(CC):
                        nc.tensor.matmul(
                            ps[:, 0:GPC * HD],
                            (x_sb[:, cc * T + tb * 128:
                                    cc * T + (tb + 1) * 128]),
                            (wv_sb[:, cc * GPC * HD:(cc + 1) * GPC * HD]),
                            start=(cc == 0), stop=(cc == CC - 1))
                    nc.vector.tensor_copy(
                        vp_v[:, tb, :, 0:64],
                        ps[:, 0:GPC * HD].rearrange("p (g k) -> p g k", k=64))
                    drain(2)

                def emit_qg(gi):
                    qg_sb = qg_pool.tile([128, CC * T],
                                         BF16 if s_bf16 else F32R, tag="qg",
                                         name=f"qg{gi}")
                    for cc in range(CC):
                        nc.vector.tensor_scalar_mul(
                            qg_sb[:, cc * T:(cc + 1) * T],
                            q_sb[:, cc * T:(cc + 1) * T],
                            al_sb[:, cc * GPC + gi:cc * GPC + gi + 1])
                    return qg_sb

                def emit_av(gi, tb, avs, e_t):
                    for qh in range(2):
                        # rows 0-63: v @ E; row 64: rowsum of E
                        nc.tensor.matmul(
                            avs[qh][0:VG, :],
                            vp_v[:, tb, gi, 0:VG],
                            e_t[:, qh * 512:(qh + 1) * 512],
                            start=(tb == 0), stop=(tb == TB - 1))

                def emit_sq_pe(tb, sqs, e2_t):
                    one_col = onb_sb[:, 0:1] if av_bf16 else ones_sb[:, 0:1]
                    for qh in range(2):
                        nc.tensor.matmul(
                            sqs[qh][0:1, :],
                            one_col,
                            e2_t[:, qh * 512:(qh + 1) * 512],
                            start=(tb == 0), stop=(tb == TB - 1))

                def emit_sq(chains):
                    # contract the accumulated E^2 chains: PSUM-accumulate
                    # one ones-matmul per chain per qh-half.
                    sqs = tuple(av_ps.tile([128, 512], F32, tag="sq",
                                           name=f"sq{qh}") for qh in range(2))
                    one_col = onb_sb[:, 0:1] if av_bf16 else ones_sb[:, 0:1]
                    for qh in range(2):
                        for ci, e2a in enumerate(chains):
                            nc.tensor.matmul(
                                sqs[qh][0:1, :],
                                one_col,
                                e2a[:, qh * 512:(qh + 1) * 512],
                                start=(ci == 0), stop=(ci == len(chains) - 1))
                    return sqs

                def emit_finish(gi, avs, sqs, g_sb):
                    # AV PSUM -> SBUF (rows 0..63 only; rowsum row staged for
                    # stats on Pool and Ln-read straight from PSUM).
                    stat_sb = st_pool.tile([1, 2048], F32, tag="stat",
                                           name="stat_sb", bufs=2)
                    avc = []
                    for qh in range(2):
                        a_sb = st_pool.tile([128, 512], F32, tag="avc",
                                            name="a_sb", bufs=3)
                        nc.vector.tensor_copy(a_sb[0:64, :],
                                              avs[qh][0:64, :])
                        avc.append(a_sb)
                        # raw stats rows: rowsum + sqsum (DVE; Pool lacks
                        # PSUM access)
                        nc.vector.tensor_copy(
                            stat_sb[0:1, qh * 512:(qh + 1) * 512],
                            avs[qh][64:65, :])
                        nc.vector.tensor_copy(
                            stat_sb[0:1, 1024 + qh * 512:1536 + qh * 512],
                            sqs[qh][0:1, :])
                    nc.sync.dma_start(st_d[gi:gi + 1, :], stat_sb[:])
                    for qh in range(2):
                        # r = 1/rowsum via exp(-ln .) on ACT, broadcast to
                        # partitions 0..63 with a 1-row fp32 matmul.
                        lnr = st_pool.tile([128, 512], F32, tag="lnr",
                                           name="lnr")
                        nc.scalar.activation(lnr[64:65, :], avs[qh][64:65, :],
                                             mybir.ActivationFunctionType.Ln)
                        rrow = st_pool.tile([128, 512],
                                            BF16 if rb_bf16 else F32,
                                            tag="rrow", name="rrow")
                        nc.scalar.activation(rrow[64:65, :], lnr[64:65, :],
                                             mybir.ActivationFunctionType.Exp,
                                             scale=-1.0)
                        rb_ps = av_ps.tile([128, 512], F32, tag="sq",
                                           name="rb_ps")
                        if rb_bf16:
                            nc.tensor.matmul(
                                rb_ps[0:64, :],
                                onb_sb[64:65, 0:64],
                                rrow[64:65, :],
                                start=True, stop=True)
                        else:
                            nc.tensor.matmul(
                                rb_ps[0:64, :],
                                ones_sb[64:65, 0:64].bitcast(F32),
                                rrow[64:65, :],
                                start=True, stop=True)
                        # G = (v@E) * r
                        if proj2:
                            # stacked-pair layout: g_sb[128, 1024],
                            # g_sb[64*pi + d, m*128 + i] = G[d, 8i+2m+pi]
                            gv = g_sb[:].rearrange("p (m i) -> p m i",
                                                   m=4)
                            for pi in range(2):
                                src_ap = (avc[qh][0:64, :]
                                          .rearrange("p (i m) -> p m i",
                                                     m=8)[:, pi::2, :])
                                rb_ap = (rb_ps[0:64, :]
                                         .rearrange("p (i m) -> p m i",
                                                    m=8)[:, pi::2, :])
                                nc.vector.tensor_tensor(
                                    gv[pi * 64:(pi + 1) * 64, :,
                                       qh * 64:(qh + 1) * 64],
                                    src_ap, rb_ap,
                                    mybir.AluOpType.mult)
                        else:
                            nc.vector.tensor_tensor(
                                g_sb[:, qh * 512:(qh + 1) * 512],
                                avc[qh][0:64, :], rb_ps[0:64, :],
                                mybir.AluOpType.mult)

                def emit_proj(gi, g_sb):
                    # "sq" ring, not "av": sharing the 2-slot av ring with the
                    # map-lifetime AV accumulators would make map g's proj
                    # wait on map g+1's whole AV stream (slot clobber-wait).
                    p_ps = av_ps.tile([128, 512], F32, tag="sq", name="p_ps")
                    if proj2:
                        for m in range(4):
                            nc.tensor.matmul(
                                p_ps[:], g_sb[:, m * 128:(m + 1) * 128],
                                wp_sb[:, m * C:(m + 1) * C],
                                start=(m == 0), stop=(m == 3))
                    else:
                        g_v = g_sb[:].rearrange("p (i s) -> p s i", s=8)
                        for jh in range(8):
                            nc.tensor.matmul(p_ps[:], g_v[:, jh, :],
                                             wp_sb[:, jh * C:(jh + 1) * C],
                                             start=(jh == 0), stop=(jh == 7))
                    stage_t = st_pool.tile([128, 512], F32, tag="stage",
                                           name="stage", bufs=2)
                    nc.scalar.copy(stage_t[:], p_ps[:])
                    nc.sync.dma_start(out_d[gi * 128:(gi + 1) * 128, :],
                                      stage_t[:])

                qg_next = emit_qg(0)
                for gi in range(GPC):
                    qg_sb = qg_next
                    avs = tuple(av_ps.tile([128, 512], F32, tag="av",
                                           name=f"av{qh}") for qh in range(2))
                    if e2_acc == "pe":
                        sqs = tuple(av_ps.tile([128, 512], F32, tag="sq",
                                               name=f"sq{qh}")
                                    for qh in range(2))
                        chains = None
                    else:
                        nchain = 2 if e2_acc == "split" else 1
                        chains = [e2_pool.tile([128, 1024],
                                               BF16 if av_bf16 else F32R,
                                               tag="e2a",
                                               name=f"e2a{ci}", bufs=2)
                                  for ci in range(nchain)]
                        sqs = None

                    for tb in range(TB):
                        if tb == 4 and gi + 1 < GPC:
                            qg_next = emit_qg(gi + 1)
                        if lvl < 1:
                            continue
                        s_ps = qk_ps.tile([128, 1024], F32, tag="mmps",
                                          name="s_ps")
                        if wide:
                            for oc in range(CC):
                                nc.tensor.matmul(
                                    s_ps[:, 0:1024],
                                    k_sb[:, oc * T + tb * 128:
                                         oc * T + (tb + 1) * 128],
                                    qg_sb[:, oc * T:oc * T + 1024],
                                    start=(oc == 0), stop=(oc == CC - 1))
                        else:
                            s_order = ([(qh, oc) for oc in range(CC)
                                        for qh in range(2)] if reorder else
                                       [(qh, oc) for qh in range(2)
                                        for oc in range(CC)])
                            for qh, oc in s_order:
                                nc.tensor.matmul(
                                    s_ps[:, qh * 512:(qh + 1) * 512],
                                    k_sb[:, oc * T + tb * 128:
                                         oc * T + (tb + 1) * 128],
                                    qg_sb[:, oc * T + qh * 512:
                                          oc * T + qh * 512 + 512],
                                    start=(oc == 0), stop=(oc == CC - 1))
                        if lvl < 2:
                            continue
                        e_t = e_pool.tile([128, 1024], av_dt)
                        nc.scalar.activation(e_t[:], s_ps[:],
                                             mybir.ActivationFunctionType.Exp)
                        if lvl < 3:
                            continue
                        # ---- E^2 ----
                        use_act = (e2_sq == "act" or
                                   (e2_sq == "alt" and tb % 2 == 1))
                        e2_t = e2_pool.tile([128, 1024],
                                            BF16 if av_bf16 else F32R)
                        if use_act:
                            nc.scalar.activation(
                                e2_t[:], s_ps[:],
                                mybir.ActivationFunctionType.Exp, scale=2.0)
                        else:
                            nc.vector.tensor_mul(e2_t[:], e_t[:], e_t[:])
                        if e2_acc == "pe":
                            pass  # handled with the AV todo below
                        else:
                            ci = tb % len(chains)
                            eng = (nc.vector if (e2_acc == "dve" or
                                                 (e2_acc == "split" and
                                                  ci == 0))
                                   else nc.gpsimd)
                            acc = chains[ci]
                            if tb < len(chains):
                                eng.tensor_copy(acc[:], e2_t[:])
                            else:
                                eng.tensor_add(acc[:], acc[:], e2_t[:])
                        if lvl < 4:
                            continue
                        if e2_acc == "pe":
                            todo.append(lambda gi=gi, tb=tb, a=avs, e=e_t,
                                        s=sqs, e2=e2_t:
                                        (emit_av(gi, tb, a, e),
                                         emit_sq_pe(tb, s, e2)))
                        else:
                            todo.append(lambda gi=gi, tb=tb, a=avs, e=e_t:
                                        emit_av(gi, tb, a, e))
                        while len(todo) > depth:
                            todo.popleft()()
                    if lvl < 5:
                        continue
                    g_sb = g_pool.tile([128, 512] if proj2
                                       else [64, 1024], F32R)
                    if e2_acc == "pe":
                        todo.append(lambda gi=gi, a=avs, s=sqs, g=g_sb:
                                    emit_finish(gi, a, s, g))
                    else:
                        todo.append(lambda gi=gi, a=avs, ch=chains, g=g_sb:
                                    emit_finish(gi, a, emit_sq(ch), g))
                    todo.append(lambda gi=gi, g=g_sb: emit_proj(gi, g))
                if not overlap:
                    while todo:
                        todo.popleft()()
            while todo:
                todo.popleft()()

    _pin_act_table(nc)
    nc.compile()
    _prog_cache[key] = nc
    return nc


def _pin_act_table(nc):
    """Keep one act-table set (Exp/Ln/Copy) resident; no per-map thrash."""
    import bass_rust as _bass_rust
    from concourse.hw_specs import get_activation_tables
    keep = "natural_log_exp_and_others"
    af = mybir.ActivationFunctionType
    ours = {af.Exp, af.Ln, af.Copy, af.Identity}

    def patched_pass():
        has_activation = any(
            isinstance(i, mybir.InstActivation)
            for b in nc.main_func.blocks for i in b.instructions)
        if not has_activation:
            return
        tables = get_activation_tables(nc.m.arch)
        if keep in tables and ours <= set(tables[keep]):
            tables = {name: (fns if name == keep else set(fns) - ours)
                      for name, fns in tables.items()}
        _bass_rust.insert_act_table_loads(nc, list(tables.items()))

    nc.insert_act_table_loads = patched_pass


def _host_prep(x, w_q, w_k, w_v, w_head, in_gamma, in_beta, w_proj, b_proj):
    x = np.asarray(x, dtype=np.float32)
    w_q = np.asarray(w_q, dtype=np.float32)
    w_k = np.asarray(w_k, dtype=np.float32)
    w_v = np.asarray(w_v, dtype=np.float32)
    w_head = np.asarray(w_head, dtype=np.float32)

    wqT = np.ascontiguousarray(w_q.T) * np.float32(SCALE)
    wkT = np.ascontiguousarray(w_k.T)
    wpT_r = np.ascontiguousarray(
        np.asarray(w_proj, dtype=np.float32)
        .T.reshape(8, 64, C).transpose(1, 0, 2).reshape(64, 8 * C))
    # stacked-pair layout: wpT2[64*pi + d, m*C + c] = wpT_r[d, (2m+pi)*C + c]
    wpT2 = np.empty((128, 4 * C), dtype=np.float32)
    for m in range(4):
        wpT2[0:64, m * C:(m + 1) * C] = wpT_r[:, (2 * m) * C:(2 * m + 1) * C]
        wpT2[64:128, m * C:(m + 1) * C] = (
            wpT_r[:, (2 * m + 1) * C:(2 * m + 2) * C])

    in_maps = []
    p = np.arange(128)
    for core in range(NCORES):
        b = core // 2
        g0 = (core % 2) * GPC
        xc = np.ascontiguousarray(x[b].reshape(C, T))
        wvT = np.ascontiguousarray(w_v.T[:, g0 * HD:(g0 + GPC) * HD])
        al = np.empty((128, CC * GPC), dtype=np.float32)
        for cc in range(CC):
            for gi in range(GPC):
                al[:, cc * GPC + gi] = w_head[g0 + gi, cc * 2 + p // 64]
        import ml_dtypes
        in_maps.append({
            "x": xc, "wqT": wqT, "wkT": wkT, "wvT": wvT,
            "wpT": wpT_r, "alphas": al,
            "ones": np.ones((128, 128), dtype=np.float32),
            "onesb": np.ones((128, 64), dtype=ml_dtypes.bfloat16),
            "wpT2": wpT2,
        })
    return in_maps


def _host_finish(results, x, w_v, w_head, in_gamma, in_beta, w_proj, b_proj):
    in_gamma = np.asarray(in_gamma, dtype=np.float32)
    in_beta = np.asarray(in_beta, dtype=np.float32)
    w_proj = np.asarray(w_proj, dtype=np.float32)
    b_proj = np.asarray(b_proj, dtype=np.float32)
    w_v = np.asarray(w_v, dtype=np.float32)
    x = np.asarray(x, dtype=np.float32)

    collapsed_wp = w_proj.reshape(C, 8, 64).sum(axis=1).T   # [64, C]
    out = np.empty((B, C, T), dtype=np.float32)
    for core in range(NCORES):
        b = core // 2
        g0 = (core % 2) * GPC
        dev = results[core]["out"]                 # [512 i, 512 c]
        stats = results[core]["stats"]             # [GPC*4, 512]
        sv = w_v @ x[b].reshape(C, T).sum(axis=1)  # [C]
        for gi in range(GPC):
            g = g0 + gi
            rowsum = stats[gi, 0:1024].astype(np.float64)
            sqsum = stats[gi, 1024:2048].astype(np.float64)
            s2 = float(np.sum(sqsum / (rowsum * rowsum)))
            var = s2 / float(T * T) - MU * MU
            a = in_gamma[g] / np.sqrt(var + EPS)
            cs = in_beta[g] - a * MU
            bias2 = collapsed_wp.T @ sv[g * HD:(g + 1) * HD]   # [C]
            blk = dev[gi * 128:(gi + 1) * 128, :]              # [128 i, C]
            full = a * blk + (cs * bias2 + b_proj)[None, :]
            out[b, :, g * 128:(g + 1) * 128] = full.T
    return out.reshape(B, C, HH, WW)


_runner_cache = {}


def _make_runner(nc):
    if id(nc) in _runner_cache:
        return _runner_cache[id(nc)]

    from concourse import bass2jax as b2j
    import concourse.mybir as _mybir

    b2j.install_neuronx_cc_hook()
    partition_name = (nc.partition_id_tensor.name
                      if nc.partition_id_tensor else None)
    in_names, out_names, out_avals, zero_shapes = [], [], [], []
    for alloc in nc.m.functions[0].allocations:
        if not isinstance(alloc, _mybir.MemoryLocationSet):
            continue
        name = alloc.memorylocations[0].name
        if alloc.kind == "ExternalInput":
            if name != partition_name:
                in_names.append(name)
        elif alloc.kind == "ExternalOutput":
            shape = tuple(alloc.tensor_shape)
            dtype = _mybir.dt.np(alloc.dtype)
            out_names.append(name)
            out_avals.append(jax.core.ShapedArray(shape, dtype))
            zero_shapes.append((shape, dtype))
    n_params = len(in_names)
    all_names = in_names + out_names + (
        [partition_name] if partition_name else [])

    def _body(*args):
        operands = list(args)
        if partition_name is not None:
            operands.append(b2j.partition_id_tensor())
        return tuple(b2j._bass_exec_p.bind(
            *operands, out_avals=tuple(out_avals), in_names=tuple(all_names),
            out_names=tuple(out_names), lowering_input_output_aliases=(),
            sim_require_finite=True, sim_require_nnan=True, nc=nc))

    from jax.experimental.shard_map import shard_map
    from jax.sharding import Mesh, NamedSharding, PartitionSpec
    devices = jax.devices()[:NCORES]
    mesh = Mesh(np.asarray(devices), ("core",))
    nin = n_params + len(out_names)
    sharded = jax.jit(
        shard_map(_body, mesh=mesh, in_specs=(PartitionSpec("core"),) * nin,
                  out_specs=(PartitionSpec("core"),) * len(out_names),
                  check_rep=False),
        keep_unused=True)
    shard = NamedSharding(mesh, PartitionSpec("core"))
    dev_cache = {}

    def run(in_maps):
        key = id(in_maps)
        if dev_cache.get("key") != key:
            concat_in = [
                np.concatenate([np.asarray(m[name]) for m in in_maps], axis=0)
                for name in in_names]
            dev_cache["in"] = [jax.device_put(a, shard) for a in concat_in]
            dev_cache["zero"] = [
                jax.device_put(np.zeros((NCORES * s[0], *s[1:]), d), shard)
                for s, d in zero_shapes]
            dev_cache["key"] = key
        outs = sharded(*dev_cache["in"], *dev_cache["zero"])
        return [
            {name: np.asarray(outs[i]).reshape(NCORES, *out_avals[i].shape)[c]
             for i, name in enumerate(out_names)}
            for c in range(NCORES)]

    _runner_cache[id(nc)] = run
    return run


def _run(inputs, trace=False, reps=1, **kw):
    nc = build_program(reps, **kw)
    in_maps = _host_prep(**inputs)
    results = _make_runner(nc)(in_maps)
    out = _host_finish(results, inputs["x"], inputs["w_v"],
                       inputs["w_head"], inputs["in_gamma"],
                       inputs["in_beta"], inputs["w_proj"], inputs["b_proj"])
    return out, results


def kernel(**inputs):
    out, _ = _run(inputs, trace=False)
    return out
